# revision 37
# baseline (speedup 1.0000x reference)
"""DendriticFullyConnected Trainium2 kernel — mixed bf16 / fp8-DoubleRow.

Math (per reference):
  x_c  = x[:, :409];  x_nc = x[:, 409:]
  state = sigmoid(x_nc @ W_non.T + b_non) - 1
  cluster = (x_c * coeff) @ W_nmda.T          # coeff = [1,2,...,2,1]
  pre = cluster + state
  out = pre^2 / (0.25 + pre^2)

Strategy: data-parallel over batch on 8 cores (1024 rows each), weights
replicated.  The contraction splits by precision sensitivity:

  nmda part (K=409->512, 4 k-tiles)  : bf16.  cluster hits the Hill directly
    (sigma~2, gain ~1), so fp8 here costs ~5e-2 rel err.  bf16 keeps it at
    ~3e-3 and runs at 1 cycle/row (216 ns per [128k,128o]x[128k,512b] MM).
  non part (K=3687+bias->3840, 15 pairs of k-tiles): fp8 e4m3 with
    perf_mode=DoubleRow (2 fp8 weights per PE cell -> 256-deep contraction
    per 216 ns matmul = 2x bf16 FLOPs; the DR LDWEIGHTS (135 ns) hides
    behind the previous matmul).  The sigmoid's <=0.25 gain squashes the
    fp8 quantization noise (measured 6.4e-3 rel-l2 end to end vs the 2e-2
    gate).  W_non/b_non are pre-scaled by 64 so sigma~1 lands mid e4m3
    range (away from subnormals); 1/64 is folded into the sigmoid's scale.

Layouts are all host-prepared so every DMA is a straight contiguous copy:
  xnm [512, 1024/core] bf16;  xnn [15kp*128p, 2i*1024b] fp8 (i = DoubleRow
  half, logical k = kp*256 + i*128 + p);  wnm rows ot*128+p, cols kt*128+o;
  wnn rows ot*128+p, cols kp*256 + i*128 + o.  Bias rides as x-row 3687
  (ones) paired with b_non*64 in wnn.

Device: outT[o, b] = sum_k wt[k, o] xt[k, b] with W-stationary matmuls
(lhsT = w tile, rhs = cached x), two PSUM groups (nmda / non) per o-tile,
then the sigmoid + Hill epilogue on ACT/DVE — sigmoid(ACT, psum-read,
scale folds the fp8 prescale), pre=nm-sig (DVE), pre^2 (ACT), den=sq+KD,
rec=1/den, out=1-KD*rec (DVE; a true DVE divide would save one op but
walrus codegen rejects AluOpType.divide).  Output bf16; host upcasts.

Scheduling: phase A runs the bf16 nmda phases of the first OT_AHEAD o-tiles
k-OUTER in groups of 4 (psum budget) so each arriving xm k-tile unlocks
8 matmuls while the x fill streams.  The startup fill runs at the
per-core HBM roofline (~6 MB of phase-A-critical traffic), so the Sync
queue issues the ten 131 KB wm tiles FIRST (needed from ~10 us), then the
wn0/wn1 prefetches, then the remaining even-kp xnn; odd-kp xnn and xm ride
GpSimd/ACT.  Phases B/C are the plain o-outer loop.  Output stores ride the GpSimd SWDGE queue (a
data-blocked store trigger on the ACT queue would head-of-line-block the
epilogue stream: ~25 us of tail + psum-WAR stalls); only the last two
o-tiles store via ACT/HWDGE so the slow SWDGE completion drain (~7 us)
leaves the teardown's critical path.
"""

import numpy as np
import ml_dtypes

B = 8192
IN_F = 4096
OUT_F = 4096
IC = 409                      # clustering synapses
INC = IN_F - IC               # 3687
KD = 0.25                     # Hill k_d = k_a^n = 0.5^2
NCORES = 8
BLOC = B // NCORES            # 1024
OT = OUT_F // 128             # 32 output-row tiles
NBH = BLOC // 512             # 2 batch halves (512 = max matmul free dim)
OT_AHEAD = 10                 # o-tiles whose nmda phase covers the x fill

KNM_PAD = 512                 # nmda contraction, padded (4 k-tiles, bf16)
KNM_TILES = 4
KNN = INC + 1                 # 3688: non contraction + bias row
KP = 15                       # fp8 DoubleRow k-pairs (15 * 256 = 3840)
KNN_PAD = KP * 256
S_W = 64.0                    # fp8 pre-scale on W_non/b_non

_nc_cache = []


def _build():
    import concourse.bacc as bacc
    import concourse.tile as tile
    import concourse.mybir as mybir

    f32 = mybir.dt.float32
    bf16 = mybir.dt.bfloat16
    f8 = mybir.dt.float8e4
    ACT = mybir.ActivationFunctionType
    DR = mybir.MatmulPerfMode.DoubleRow

    nc = bacc.Bacc(None, target_bir_lowering=False)
    xnm = nc.dram_tensor("xnm", [KNM_PAD, BLOC], bf16, kind="ExternalInput")
    xnn = nc.dram_tensor("xnn", [KP * 128, 2 * BLOC], f8, kind="ExternalInput")
    wnm = nc.dram_tensor("wnm", [OUT_F, KNM_PAD], bf16, kind="ExternalInput")
    wnn = nc.dram_tensor("wnn", [OUT_F, KP * 256], f8, kind="ExternalInput")
    outT = nc.dram_tensor("outT", [OUT_F, BLOC], bf16, kind="ExternalOutput")

    with tile.TileContext(nc) as tc:
        with (
            tc.tile_pool(name="xpool", bufs=1) as xpool,
            tc.tile_pool(name="wmpool", bufs=36) as wmpool,
            tc.tile_pool(name="wnpool", bufs=4) as wnpool,
            tc.tile_pool(name="nmpool", bufs=24) as nmpool,
            tc.tile_pool(name="tmp", bufs=8) as tmp,
            tc.tile_pool(name="opool", bufs=8) as opool,
            tc.tile_pool(name="psum", bufs=8, space="PSUM") as psum,
        ):
            def osl(ot):
                return slice(ot * 128, (ot + 1) * 128)

            def bsl(bh):
                return slice(bh * 512, (bh + 1) * 512)

            x_pending = []

            def feed_x(n):
                for _ in range(n):
                    if x_pending:
                        t, src = x_pending.pop()
                        nc.sync.dma_start(t[:], src)

            wm_tiles = {}

            def prefetch_wm(ot):
                # four per-kt tiles, not one: a tile's DMA semaphore stays
                # live until its LAST reader, and a whole-ot wm tile is read
                # across the full 6.9 us k-outer group sweep — with the ~8
                # DMA-semaphore slots that serialized the next group's wm
                # fetches behind the current group (a ~4 us PE gap).  Per-kt
                # tiles free their slot after one matmul pair.
                if ot not in wm_tiles:
                    ts = []
                    for kt in range(KNM_TILES):
                        wg = wmpool.tile(
                            [128, 128], bf16, tag="wm", name=f"wm_{ot}_{kt}"
                        )
                        nc.sync.dma_start(
                            wg[:], wnm[osl(ot), kt * 128 : (kt + 1) * 128]
                        )
                        ts.append(wg)
                    wm_tiles[ot] = ts

            def get_wm(ot):
                prefetch_wm(ot)
                return wm_tiles.pop(ot)

            wn_tiles = {}

            def prefetch_wn(ot):
                if ot not in wn_tiles:
                    wg = wnpool.tile([128, KP, 2, 128], f8, tag="wn", name=f"wn_{ot}")
                    nc.sync.dma_start(
                        wg[:],
                        wnn[osl(ot), :].rearrange("p (k i o) -> p k i o", k=KP, i=2),
                    )
                    wn_tiles[ot] = wg

            def get_wn(ot):
                prefetch_wn(ot)
                return wn_tiles.pop(ot)

            # ── x cache fill ────────────────────────────────────────────
            # (see module docstring).
            # xm tiles one-per-queue so each lands before its k-outer
            # sweep slot (all-on-GpSimd serialized ~0.9 MB behind SWDGE
            # completion lag and starved groups 1-2 for ~4 us):
            #   xm0 split scalar+gpsimd (gates the first matmul),
            #   xm1 scalar, xm2 gpsimd, xm3 sync (issued after wm3 below).
            xm = []
            xm3_src = None
            for kt in range(KNM_TILES):
                t = xpool.tile([128, BLOC], bf16, tag=f"xm{kt}")
                src = xnm[kt * 128 : (kt + 1) * 128, :]
                if kt == 0:
                    nc.scalar.dma_start(t[:, 0:512], src[:, 0:512])
                    nc.gpsimd.dma_start(t[:, 512:], src[:, 512:])
                elif kt == 1:
                    nc.scalar.dma_start(t[:], src)
                elif kt == 2:
                    nc.gpsimd.dma_start(t[:], src)
                else:
                    xm3_src = (t, src)
                xm.append(t)
            xn = []
            for kp in range(KP):
                t = xpool.tile([128, 2, BLOC], f8, tag=f"xn{kp}")
                src = xnn[kp * 128 : (kp + 1) * 128, :].rearrange(
                    "p (i b) -> p i b", i=2
                )
                if kp % 2 == 1:
                    nc.gpsimd.dma_start(t[:], src)
                elif kp >= 12:
                    nc.scalar.dma_start(t[:], src)
                else:
                    x_pending.append((t, src))
                xn.append(t)
            x_pending.reverse()  # pop() from the front of the schedule

            def nmda_group(ots):
                # k-OUTER over a group of o-tiles (<=4: psum budget): during
                # the x fill each arriving xm[kt] unlocks len(ots)*2 matmuls
                # instead of 2, keeping the PE fed while xnm streams in.
                wgs = [get_wm(ot) for ot in ots]
                psn = [
                    [
                        psum.tile([128, 512], f32, tag="ps", name=f"psn_{ot}_{i}")
                        for i in range(NBH)
                    ]
                    for ot in ots
                ]
                for kt in range(KNM_TILES):
                    for j in range(len(ots)):
                        for bh in range(NBH):
                            nc.tensor.matmul(
                                psn[j][bh][:],
                                lhsT=wgs[j][kt][:],
                                rhs=xm[kt][:, bsl(bh)],
                                start=(kt == 0),
                                stop=(kt == KNM_TILES - 1),
                            )
                nms = []
                for j, ot in enumerate(ots):
                    nm = []
                    for bh in range(NBH):
                        t = nmpool.tile([128, 512], f32, tag="nm", name=f"nm_{ot}_{bh}")
                        nc.scalar.copy(t[:], psn[j][bh][:])
                        nm.append(t)
                    nms.append(nm)
                return nms

            def nmda_phase(ot):
                return nmda_group([ot])[0]

            def non_phase(ot, bh_outer=False):
                wg = get_wn(ot)
                ps = [
                    psum.tile([128, 512], f32, tag="ps", name=f"ps_{ot}_{i}")
                    for i in range(NBH)
                ]
                # bh_outer (last o-tile): bh0's accumulation stops ~3.2 us
                # before the final matmul, so its epilogue chain overlaps
                # bh1's matmuls and only bh1's chain trails the kernel.
                order = (
                    [(kp, bh) for bh in range(NBH) for kp in range(KP)]
                    if bh_outer
                    else [(kp, bh) for kp in range(KP) for bh in range(NBH)]
                )
                for kp, bh in order:
                    nc.tensor.matmul(
                        ps[bh][:],
                        lhsT=wg[:, kp, :, :],
                        rhs=xn[kp][:, :, bsl(bh)],
                        start=(kp == 0),
                        stop=(kp == KP - 1),
                        perf_mode=DR,
                    )
                return ps

            def epilogue_pair(ot, ps_pair, nm_pair, nch=1):
                # psum = S_W*(z+b); pre = nm - sigmoid(-(z+b));
                # out = pre^2/(KD+pre^2) = 1 - KD/(KD+pre^2).  Chains
                # interleaved so ACT and DVE overlap across the batch halves.
                # nch>1 runs the chain on column chunks so the last o-tile's
                # epilogue pipelines instead of paying the full serial chain.
                sig = [
                    tmp.tile([128, 512], f32, tag="t", name=f"sig_{ot}_{bh}")
                    for bh in range(NBH)
                ]
                rec = [
                    tmp.tile([128, 512], f32, tag="t", name=f"rec_{ot}_{bh}")
                    for bh in range(NBH)
                ]
                ob = [
                    opool.tile([128, 512], bf16, tag="o", name=f"ob_{ot}_{bh}")
                    for bh in range(NBH)
                ]
                cw = 512 // nch

                def csl(c):
                    return slice(c * cw, (c + 1) * cw)

                for c in range(nch):
                    for bh in range(NBH):
                        nc.scalar.activation(
                            sig[bh][:, csl(c)],
                            ps_pair[bh][:, csl(c)],
                            ACT.Sigmoid,
                            scale=-1.0 / S_W,
                        )
                    for bh in range(NBH):
                        nc.vector.tensor_sub(
                            sig[bh][:, csl(c)], nm_pair[bh][:, csl(c)], sig[bh][:, csl(c)]
                        )  # := pre
                    for bh in range(NBH):
                        nc.scalar.activation(
                            nm_pair[bh][:, csl(c)], sig[bh][:, csl(c)], ACT.Square
                        )
                    for bh in range(NBH):
                        nc.vector.tensor_scalar_add(
                            sig[bh][:, csl(c)], nm_pair[bh][:, csl(c)], KD
                        )
                    for bh in range(NBH):
                        nc.vector.reciprocal_approx_fast(
                            rec[bh][:, csl(c)], sig[bh][:, csl(c)]
                        )
                    for bh in range(NBH):
                        nc.vector.tensor_scalar(
                            ob[bh][:, csl(c)], rec[bh][:, csl(c)], -KD, 1.0,
                            mybir.AluOpType.mult, mybir.AluOpType.add,
                        )
                for bh in range(NBH):
                    if ot >= OT - 2:
                        # Sync is idle once the W stream ends; a trigger on
                        # ACT would cost ~0.6 us each inside the final
                        # epilogue's ACT chain.
                        nc.sync.dma_start(outT[osl(ot), bsl(bh)], ob[bh][:])
                    else:
                        nc.gpsimd.dma_start(outT[osl(ot), bsl(bh)], ob[bh][:])

            # ── Phase A: nmda for the first OT_AHEAD o-tiles (xnm only).
            # Sync queue order: all 10 wm tiles (131 KB each, needed from
            # ~10 us on) BEFORE the big wn prefetches and the x evens —
            # a 256 KB feed between wm DMAs delays every later wm by ~1 us
            # and was starving the group-1/2 nmda sweeps. ──
            for ot in range(OT_AHEAD):
                prefetch_wm(ot)
                if ot == 3 and xm3_src is not None:
                    nc.sync.dma_start(xm3_src[0][:], xm3_src[1])
            prefetch_wn(0)
            prefetch_wn(1)
            feed_x(len(x_pending))
            nm_ahead = []
            groups = [
                list(range(g, min(g + 4, OT_AHEAD))) for g in range(0, OT_AHEAD, 4)
            ]
            for grp in groups:
                nm_ahead.extend(nmda_group(grp))

            # ── Phase B: non + epilogue for the ahead o-tiles ──
            for ot in range(OT_AHEAD):
                ps = non_phase(ot)
                epilogue_pair(ot, ps, nm_ahead[ot])

            # ── Phase C: remaining o-tiles, plain o-outer loop ──
            for ot in range(OT_AHEAD, OT):
                nm = nmda_phase(ot)
                ps = non_phase(ot, bh_outer=(ot == OT - 1))
                epilogue_pair(ot, ps, nm)
    nc.compile()
    return nc


def _warmup():
    """Tiny throwaway NEFF run: the first execution after session start
    occasionally dies with NRT_EXEC_UNIT_UNRECOVERABLE; absorb that here."""
    import concourse.bacc as bacc
    import concourse.tile as tile
    import concourse.mybir as mybir
    from concourse.bass_utils import run_bass_kernel_spmd

    nc = bacc.Bacc(None, target_bir_lowering=False)
    a = nc.dram_tensor("a", [128, 128], mybir.dt.float32, kind="ExternalInput")
    b = nc.dram_tensor("b", [128, 128], mybir.dt.float32, kind="ExternalOutput")
    with tile.TileContext(nc) as tc:
        with tc.tile_pool(name="p", bufs=1) as pool:
            t = pool.tile([128, 128], mybir.dt.float32)
            nc.sync.dma_start(t[:], a[:])
            nc.sync.dma_start(b[:], t[:])
    nc.compile()
    ins = [{"a": np.zeros((128, 128), np.float32)} for _ in range(NCORES)]
    for _ in range(3):
        try:
            run_bass_kernel_spmd(nc, ins, core_ids=list(range(NCORES)))
            return
        except Exception:
            continue


def kernel(x, W_nmda, W_non, b_non):
    from concourse.bass_utils import run_bass_kernel_spmd

    x = np.asarray(x, dtype=np.float32)
    W_nmda = np.asarray(W_nmda, dtype=np.float32)
    W_non = np.asarray(W_non, dtype=np.float32)
    b_non = np.asarray(b_non, dtype=np.float32)

    coeff = np.full((IC,), 2.0, dtype=np.float32)
    coeff[0] = 1.0
    coeff[-1] = 1.0

    bf16 = ml_dtypes.bfloat16
    f8 = ml_dtypes.float8_e4m3

    # x, nmda part: [512, B] bf16
    xTm = np.zeros((KNM_PAD, B), dtype=np.float32)
    xTm[0:IC] = x[:, :IC].T
    xTm = xTm.astype(bf16)

    # x, non part: logical k = kp*256 + i*128 + p -> [15*128 rows, 2*B] fp8
    xTn = np.zeros((KNN_PAD, B), dtype=np.float32)
    xTn[0:INC] = x[:, IC:].T
    xTn[INC] = 1.0  # bias row
    xTn = (
        xTn.reshape(KP, 2, 128, B).transpose(0, 2, 1, 3).reshape(KP * 128, 2 * B)
    ).astype(f8)

    # W, nmda part: row ot*128+p, col kt*128+o, bf16
    wTm = np.zeros((KNM_PAD, OUT_F), dtype=np.float32)
    wTm[0:IC] = (W_nmda * coeff[None, :]).T
    wnm = (
        wTm.reshape(KNM_TILES, 128, OT, 128)
        .transpose(2, 1, 0, 3)
        .reshape(OUT_F, KNM_PAD)
    ).astype(bf16)

    # W, non part (pre-scaled by S_W): row ot*128+p, col kp*256+i*128+o, fp8
    wTn = np.zeros((KNN_PAD, OUT_F), dtype=np.float32)
    wTn[0:INC] = W_non.T * S_W
    wTn[INC] = b_non * S_W
    wnn = (
        wTn.reshape(KP, 2, 128, OT, 128)
        .transpose(3, 2, 0, 1, 4)
        .reshape(OUT_F, KP * 256)
    ).astype(f8)

    in_maps = [
        {
            "xnm": np.ascontiguousarray(
                xTm.reshape(KNM_PAD, NCORES, BLOC)[:, c, :]
            ),
            "xnn": np.ascontiguousarray(
                xTn.reshape(KP * 128, 2, NCORES, BLOC)[:, :, c, :].reshape(
                    KP * 128, 2 * BLOC
                )
            ),
            "wnm": wnm,
            "wnn": wnn,
        }
        for c in range(NCORES)
    ]

    if not _nc_cache:
        _warmup()
        _nc_cache.append(_build())
    nc = _nc_cache[0]

    res = None
    last_exc = None
    for _attempt in range(3):
        try:
            res = run_bass_kernel_spmd(nc, in_maps, core_ids=list(range(NCORES)))
            break
        except Exception as e:  # transient device errors (e.g. first-run NRT hiccup)
            last_exc = e
    if res is None:
        raise last_exc

    global LAST_RESULT
    LAST_RESULT = res

    out = np.empty((B, OUT_F), dtype=np.float32)
    for c in range(NCORES):
        out[c * BLOC : (c + 1) * BLOC] = res.results[c]["outT"].astype(np.float32).T
    return out


LAST_RESULT = None


# revision 39
# speedup vs baseline: 1.0508x; 1.0508x over previous
"""DendriticFullyConnected Trainium2 kernel — mixed bf16 / fp8-DoubleRow.

Math (per reference):
  x_c  = x[:, :409];  x_nc = x[:, 409:]
  state = sigmoid(x_nc @ W_non.T + b_non) - 1
  cluster = (x_c * coeff) @ W_nmda.T          # coeff = [1,2,...,2,1]
  pre = cluster + state
  out = pre^2 / (0.25 + pre^2)

Strategy: data-parallel over batch on 8 cores (1024 rows each), weights
replicated.  The contraction splits by precision sensitivity:

  nmda part (K=409->512, 4 k-tiles)  : bf16.  cluster hits the Hill directly
    (sigma~2, gain ~1), so fp8 here costs ~5e-2 rel err.  bf16 keeps it at
    ~3e-3 and runs at 1 cycle/row (216 ns per [128k,128o]x[128k,512b] MM).
  non part (K=3687+bias->3840, 15 pairs of k-tiles): fp8 e4m3 with
    perf_mode=DoubleRow (2 fp8 weights per PE cell -> 256-deep contraction
    per 216 ns matmul = 2x bf16 FLOPs; the DR LDWEIGHTS (135 ns) hides
    behind the previous matmul).  The sigmoid's <=0.25 gain squashes the
    fp8 quantization noise (measured 6.4e-3 rel-l2 end to end vs the 2e-2
    gate).  W_non/b_non are pre-scaled by 64 so sigma~1 lands mid e4m3
    range (away from subnormals); 1/64 is folded into the sigmoid's scale.

Layouts are all host-prepared so every DMA is a straight contiguous copy:
  xnm [512, 1024/core] bf16;  xnn [15kp*128p, 2i*1024b] fp8 (i = DoubleRow
  half, logical k = kp*256 + i*128 + p);  wnm rows ot*128+p, cols kt*128+o;
  wnn rows ot*128+p, cols kp*256 + i*128 + o.  Bias rides as x-row 3687
  (ones) paired with b_non*64 in wnn.

Device: outT[o, b] = sum_k wt[k, o] xt[k, b] with W-stationary matmuls
(lhsT = w tile, rhs = cached x), two PSUM groups (nmda / non) per o-tile,
then the sigmoid + Hill epilogue on ACT/DVE — sigmoid(ACT, psum-read,
scale folds the fp8 prescale), pre=nm-sig (DVE), pre^2 (ACT), den=sq+KD,
rec=1/den, out=1-KD*rec (DVE; a true DVE divide would save one op but
walrus codegen rejects AluOpType.divide).  Output bf16; host upcasts.

Scheduling: phase A runs the bf16 nmda phases of the first OT_AHEAD o-tiles
k-OUTER in groups of 4 (psum budget) so each arriving xm k-tile unlocks
8 matmuls while the x fill streams.  The startup fill runs at the
per-core HBM roofline (~6 MB of phase-A-critical traffic), so the Sync
queue issues the ten 131 KB wm tiles FIRST (needed from ~10 us), then the
wn0/wn1 prefetches, then the remaining even-kp xnn; odd-kp xnn and xm ride
GpSimd/ACT.  Phases B/C are the plain o-outer loop.  Output stores ride the GpSimd SWDGE queue (a
data-blocked store trigger on the ACT queue would head-of-line-block the
epilogue stream: ~25 us of tail + psum-WAR stalls); only the last two
o-tiles store via ACT/HWDGE so the slow SWDGE completion drain (~7 us)
leaves the teardown's critical path.
"""

import numpy as np
import ml_dtypes

B = 8192
IN_F = 4096
OUT_F = 4096
IC = 409                      # clustering synapses
INC = IN_F - IC               # 3687
KD = 0.25                     # Hill k_d = k_a^n = 0.5^2
NCORES = 8
BLOC = B // NCORES            # 1024
OT = OUT_F // 128             # 32 output-row tiles
NBH = BLOC // 512             # 2 batch halves (512 = max matmul free dim)
OT_AHEAD = 10                 # o-tiles whose nmda phase covers the x fill

KNM_PAD = 512                 # nmda contraction, padded (4 k-tiles, bf16)
KNM_TILES = 4
KNN = INC + 1                 # 3688: non contraction + bias row
KP = 15                       # fp8 DoubleRow k-pairs (15 * 256 = 3840)
KNN_PAD = KP * 256
S_W = 64.0                    # fp8 pre-scale on W_non/b_non

_nc_cache = []


def _build():
    import concourse.bacc as bacc
    import concourse.tile as tile
    import concourse.mybir as mybir

    f32 = mybir.dt.float32
    bf16 = mybir.dt.bfloat16
    f8 = mybir.dt.float8e4
    ACT = mybir.ActivationFunctionType
    DR = mybir.MatmulPerfMode.DoubleRow

    nc = bacc.Bacc(None, target_bir_lowering=False)
    xnm = nc.dram_tensor("xnm", [KNM_PAD, BLOC], bf16, kind="ExternalInput")
    xnn = nc.dram_tensor("xnn", [KP * 128, 2 * BLOC], f8, kind="ExternalInput")
    wnm = nc.dram_tensor("wnm", [OUT_F, KNM_PAD], bf16, kind="ExternalInput")
    wnn = nc.dram_tensor("wnn", [OUT_F, KP * 256], f8, kind="ExternalInput")
    outT = nc.dram_tensor("outT", [OUT_F, BLOC], bf16, kind="ExternalOutput")

    with tile.TileContext(nc) as tc:
        with (
            tc.tile_pool(name="xpool", bufs=1) as xpool,
            tc.tile_pool(name="wmpool", bufs=11) as wmpool,
            tc.tile_pool(name="wnpool", bufs=4) as wnpool,
            tc.tile_pool(name="nmpool", bufs=24) as nmpool,
            tc.tile_pool(name="tmp", bufs=8) as tmp,
            tc.tile_pool(name="opool", bufs=8) as opool,
            tc.tile_pool(name="psum", bufs=8, space="PSUM") as psum,
        ):
            def osl(ot):
                return slice(ot * 128, (ot + 1) * 128)

            def bsl(bh):
                return slice(bh * 512, (bh + 1) * 512)

            x_pending = []

            def feed_x(n):
                for _ in range(n):
                    if x_pending:
                        t, src = x_pending.pop()
                        nc.sync.dma_start(t[:], src)

            wm_tiles = {}

            def prefetch_wm(ot):
                if ot not in wm_tiles:
                    wg = wmpool.tile(
                        [128, KNM_TILES, 128], bf16, tag="wm", name=f"wm_{ot}"
                    )
                    nc.sync.dma_start(
                        wg[:],
                        wnm[osl(ot), :].rearrange("p (k o) -> p k o", k=KNM_TILES),
                    )
                    wm_tiles[ot] = wg

            def get_wm(ot):
                prefetch_wm(ot)
                return wm_tiles.pop(ot)

            wn_tiles = {}

            def prefetch_wn(ot):
                if ot not in wn_tiles:
                    wg = wnpool.tile([128, KP, 2, 128], f8, tag="wn", name=f"wn_{ot}")
                    nc.sync.dma_start(
                        wg[:],
                        wnn[osl(ot), :].rearrange("p (k i o) -> p k i o", k=KP, i=2),
                    )
                    wn_tiles[ot] = wg

            def get_wn(ot):
                prefetch_wn(ot)
                return wn_tiles.pop(ot)

            # ── x cache fill ────────────────────────────────────────────
            # (see module docstring).
            xm = []
            for kt in range(KNM_TILES):
                t = xpool.tile([128, BLOC], bf16, tag=f"xm{kt}")
                src = xnm[kt * 128 : (kt + 1) * 128, :]
                if kt == 0:
                    # split across two queues: gates the very first matmul
                    nc.scalar.dma_start(t[:, 0:512], src[:, 0:512])
                    nc.gpsimd.dma_start(t[:, 512:], src[:, 512:])
                else:
                    nc.gpsimd.dma_start(t[:], src)
                xm.append(t)
            xn = []
            for kp in range(KP):
                t = xpool.tile([128, 2, BLOC], f8, tag=f"xn{kp}")
                src = xnn[kp * 128 : (kp + 1) * 128, :].rearrange(
                    "p (i b) -> p i b", i=2
                )
                if kp % 2 == 1:
                    nc.gpsimd.dma_start(t[:], src)
                elif kp >= 12:
                    nc.scalar.dma_start(t[:], src)
                else:
                    x_pending.append((t, src))
                xn.append(t)
            x_pending.reverse()  # pop() from the front of the schedule

            def nmda_group(ots):
                # k-OUTER over a group of o-tiles (<=4: psum budget): during
                # the x fill each arriving xm[kt] unlocks len(ots)*2 matmuls
                # instead of 2, keeping the PE fed while xnm streams in.
                wgs = [get_wm(ot) for ot in ots]
                psn = [
                    [
                        psum.tile([128, 512], f32, tag="ps", name=f"psn_{ot}_{i}")
                        for i in range(NBH)
                    ]
                    for ot in ots
                ]
                for kt in range(KNM_TILES):
                    for j in range(len(ots)):
                        for bh in range(NBH):
                            nc.tensor.matmul(
                                psn[j][bh][:],
                                lhsT=wgs[j][:, kt, :],
                                rhs=xm[kt][:, bsl(bh)],
                                start=(kt == 0),
                                stop=(kt == KNM_TILES - 1),
                            )
                nms = []
                for j, ot in enumerate(ots):
                    nm = []
                    for bh in range(NBH):
                        t = nmpool.tile([128, 512], f32, tag="nm", name=f"nm_{ot}_{bh}")
                        nc.scalar.copy(t[:], psn[j][bh][:])
                        nm.append(t)
                    nms.append(nm)
                return nms

            def nmda_phase(ot):
                return nmda_group([ot])[0]

            def non_phase(ot, bh_outer=False):
                wg = get_wn(ot)
                ps = [
                    psum.tile([128, 512], f32, tag="ps", name=f"ps_{ot}_{i}")
                    for i in range(NBH)
                ]
                # bh_outer (last o-tile): bh0's accumulation stops ~3.2 us
                # before the final matmul, so its epilogue chain overlaps
                # bh1's matmuls and only bh1's chain trails the kernel.
                order = (
                    [(kp, bh) for bh in range(NBH) for kp in range(KP)]
                    if bh_outer
                    else [(kp, bh) for kp in range(KP) for bh in range(NBH)]
                )
                for kp, bh in order:
                    nc.tensor.matmul(
                        ps[bh][:],
                        lhsT=wg[:, kp, :, :],
                        rhs=xn[kp][:, :, bsl(bh)],
                        start=(kp == 0),
                        stop=(kp == KP - 1),
                        perf_mode=DR,
                    )
                return ps

            def epilogue_pair(ot, ps_pair, nm_pair, nch=1):
                # psum = S_W*(z+b); pre = nm - sigmoid(-(z+b));
                # out = pre^2/(KD+pre^2) = 1 - KD/(KD+pre^2).  Chains
                # interleaved so ACT and DVE overlap across the batch halves.
                # nch>1 runs the chain on column chunks so the last o-tile's
                # epilogue pipelines instead of paying the full serial chain.
                sig = [
                    tmp.tile([128, 512], f32, tag="t", name=f"sig_{ot}_{bh}")
                    for bh in range(NBH)
                ]
                rec = [
                    tmp.tile([128, 512], f32, tag="t", name=f"rec_{ot}_{bh}")
                    for bh in range(NBH)
                ]
                ob = [
                    opool.tile([128, 512], bf16, tag="o", name=f"ob_{ot}_{bh}")
                    for bh in range(NBH)
                ]
                cw = 512 // nch

                def csl(c):
                    return slice(c * cw, (c + 1) * cw)

                for c in range(nch):
                    for bh in range(NBH):
                        nc.scalar.activation(
                            sig[bh][:, csl(c)],
                            ps_pair[bh][:, csl(c)],
                            ACT.Sigmoid,
                            scale=-1.0 / S_W,
                        )
                    for bh in range(NBH):
                        nc.vector.tensor_sub(
                            sig[bh][:, csl(c)], nm_pair[bh][:, csl(c)], sig[bh][:, csl(c)]
                        )  # := pre
                    for bh in range(NBH):
                        nc.scalar.activation(
                            nm_pair[bh][:, csl(c)], sig[bh][:, csl(c)], ACT.Square
                        )
                    for bh in range(NBH):
                        nc.vector.tensor_scalar_add(
                            sig[bh][:, csl(c)], nm_pair[bh][:, csl(c)], KD
                        )
                    for bh in range(NBH):
                        nc.vector.reciprocal_approx_fast(
                            rec[bh][:, csl(c)], sig[bh][:, csl(c)]
                        )
                    for bh in range(NBH):
                        nc.vector.tensor_scalar(
                            ob[bh][:, csl(c)], rec[bh][:, csl(c)], -KD, 1.0,
                            mybir.AluOpType.mult, mybir.AluOpType.add,
                        )
                for bh in range(NBH):
                    if ot >= OT - 2:
                        # Sync is idle once the W stream ends; a trigger on
                        # ACT would cost ~0.6 us each inside the final
                        # epilogue's ACT chain.
                        nc.sync.dma_start(outT[osl(ot), bsl(bh)], ob[bh][:])
                    else:
                        nc.gpsimd.dma_start(outT[osl(ot), bsl(bh)], ob[bh][:])

            # ── Phase A: nmda for the first OT_AHEAD o-tiles (xnm only).
            # Sync queue order: all 10 wm tiles (131 KB each, needed from
            # ~10 us on) BEFORE the big wn prefetches and the x evens —
            # a 256 KB feed between wm DMAs delays every later wm by ~1 us
            # and was starving the group-1/2 nmda sweeps. ──
            for ot in range(OT_AHEAD):
                prefetch_wm(ot)
            prefetch_wn(0)
            prefetch_wn(1)
            feed_x(len(x_pending))
            # Only the FIRST group runs k-outer (it races the xm fill).
            # Later ahead-o-tiles run plain per-ot nmda: each consumes its
            # wm tile within 1.73 us, recycling the ~8 DMA-semaphore slots
            # at PE pace — a second k-outer group would hold 4 slots for a
            # whole 6.9 us sweep and stall its own wm fetches (~4 us gap).
            nm_ahead = []
            nm_ahead.extend(nmda_group([0, 1, 2, 3]))
            for ot in range(4, OT_AHEAD):
                nm_ahead.append(nmda_phase(ot))

            # ── Phase B: non + epilogue for the ahead o-tiles ──
            for ot in range(OT_AHEAD):
                ps = non_phase(ot)
                epilogue_pair(ot, ps, nm_ahead[ot])

            # ── Phase C: remaining o-tiles, plain o-outer loop ──
            for ot in range(OT_AHEAD, OT):
                nm = nmda_phase(ot)
                ps = non_phase(ot, bh_outer=(ot == OT - 1))
                epilogue_pair(ot, ps, nm)
    nc.compile()
    return nc


def _warmup():
    """Tiny throwaway NEFF run: the first execution after session start
    occasionally dies with NRT_EXEC_UNIT_UNRECOVERABLE; absorb that here."""
    import concourse.bacc as bacc
    import concourse.tile as tile
    import concourse.mybir as mybir
    from concourse.bass_utils import run_bass_kernel_spmd

    nc = bacc.Bacc(None, target_bir_lowering=False)
    a = nc.dram_tensor("a", [128, 128], mybir.dt.float32, kind="ExternalInput")
    b = nc.dram_tensor("b", [128, 128], mybir.dt.float32, kind="ExternalOutput")
    with tile.TileContext(nc) as tc:
        with tc.tile_pool(name="p", bufs=1) as pool:
            t = pool.tile([128, 128], mybir.dt.float32)
            nc.sync.dma_start(t[:], a[:])
            nc.sync.dma_start(b[:], t[:])
    nc.compile()
    ins = [{"a": np.zeros((128, 128), np.float32)} for _ in range(NCORES)]
    for _ in range(3):
        try:
            run_bass_kernel_spmd(nc, ins, core_ids=list(range(NCORES)))
            return
        except Exception:
            continue


def kernel(x, W_nmda, W_non, b_non):
    from concourse.bass_utils import run_bass_kernel_spmd

    x = np.asarray(x, dtype=np.float32)
    W_nmda = np.asarray(W_nmda, dtype=np.float32)
    W_non = np.asarray(W_non, dtype=np.float32)
    b_non = np.asarray(b_non, dtype=np.float32)

    coeff = np.full((IC,), 2.0, dtype=np.float32)
    coeff[0] = 1.0
    coeff[-1] = 1.0

    bf16 = ml_dtypes.bfloat16
    f8 = ml_dtypes.float8_e4m3

    # x, nmda part: [512, B] bf16
    xTm = np.zeros((KNM_PAD, B), dtype=np.float32)
    xTm[0:IC] = x[:, :IC].T
    xTm = xTm.astype(bf16)

    # x, non part: logical k = kp*256 + i*128 + p -> [15*128 rows, 2*B] fp8
    xTn = np.zeros((KNN_PAD, B), dtype=np.float32)
    xTn[0:INC] = x[:, IC:].T
    xTn[INC] = 1.0  # bias row
    xTn = (
        xTn.reshape(KP, 2, 128, B).transpose(0, 2, 1, 3).reshape(KP * 128, 2 * B)
    ).astype(f8)

    # W, nmda part: row ot*128+p, col kt*128+o, bf16
    wTm = np.zeros((KNM_PAD, OUT_F), dtype=np.float32)
    wTm[0:IC] = (W_nmda * coeff[None, :]).T
    wnm = (
        wTm.reshape(KNM_TILES, 128, OT, 128)
        .transpose(2, 1, 0, 3)
        .reshape(OUT_F, KNM_PAD)
    ).astype(bf16)

    # W, non part (pre-scaled by S_W): row ot*128+p, col kp*256+i*128+o, fp8
    wTn = np.zeros((KNN_PAD, OUT_F), dtype=np.float32)
    wTn[0:INC] = W_non.T * S_W
    wTn[INC] = b_non * S_W
    wnn = (
        wTn.reshape(KP, 2, 128, OT, 128)
        .transpose(3, 2, 0, 1, 4)
        .reshape(OUT_F, KP * 256)
    ).astype(f8)

    in_maps = [
        {
            "xnm": np.ascontiguousarray(
                xTm.reshape(KNM_PAD, NCORES, BLOC)[:, c, :]
            ),
            "xnn": np.ascontiguousarray(
                xTn.reshape(KP * 128, 2, NCORES, BLOC)[:, :, c, :].reshape(
                    KP * 128, 2 * BLOC
                )
            ),
            "wnm": wnm,
            "wnn": wnn,
        }
        for c in range(NCORES)
    ]

    if not _nc_cache:
        _warmup()
        _nc_cache.append(_build())
    nc = _nc_cache[0]

    res = None
    last_exc = None
    for _attempt in range(3):
        try:
            res = run_bass_kernel_spmd(nc, in_maps, core_ids=list(range(NCORES)))
            break
        except Exception as e:  # transient device errors (e.g. first-run NRT hiccup)
            last_exc = e
    if res is None:
        raise last_exc

    global LAST_RESULT
    LAST_RESULT = res

    out = np.empty((B, OUT_F), dtype=np.float32)
    for c in range(NCORES):
        out[c * BLOC : (c + 1) * BLOC] = res.results[c]["outT"].astype(np.float32).T
    return out


LAST_RESULT = None
